# revision 19
# baseline (speedup 1.0000x reference)
# Multi-head attention kernel for 8 Trainium2 NeuronCores.
#
# Problem: x[4,1024,1024] -> Q/K/V proj (16 heads, d=64) -> softmax(QK^T/8) ->
#          attn@V -> Wo -> Wd, returns (out[4,1024,512], attn[4,16,1024,1024]).
#
# Sharding: pure SPMD, no collectives. Core c handles batch b=c//2 and query
# rows [half*512, half*512+512) with half=c%2. Each core computes K/V for the
# full sequence of its batch (duplicated across the pair - cheaper than a
# cross-core exchange), Q only for its own rows.
#
# Dataflow (per core, all matmuls bf16 inputs, fp32 PSUM). Everything is
# feature-major ("transposed") so the TensorE contraction dim always sits on
# partitions and no on-chip transpose is ever needed:
#   QT[o,q]   = Wq @ x^T (+bq)
#   KT[o,k]   = Wk @ x^T (+bk)
#   Vones[k,·]= x @ Wv^T, interleaved with a ones column per head
#               (bv is folded into the final bias)
#   expT_h    = exp((KT_h^T @ QT_h) / 8)   [k,q] orientation, unnormalized
#   U_h[d,q] | rowsum[q] = Vones_h^T @ expT_h   (ones column -> rowsum row)
#   attn_h    = expT_h * bcast(1/rowsum)   (bf16, written [k,q]; the host
#               transposes per-core attn back to [q,k] during gather)
#   UT[o,q]   = U_h * bcast(1/rowsum)      (per-head normalized)
#   O1T[o,q]  = Wo @ UT
#   out[q,f]  = O1T^T @ Wd^T + b_final     (q-major: contiguous output DMA)
# where b_final = bd + Wd@(bo + Wo@bv) is computed on host.

import numpy as np
import ml_dtypes

B, S, E = 4, 1024, 1024
H, D = 16, 64
F = 512
NCORES = 8
P = 128
SQ = S // 2  # 512 query rows per core
EC = E // P  # 8 feature chunks
VW = D + 1   # V columns per head incl. the ones column

BF16 = ml_dtypes.bfloat16

_CACHE = {}


def _build_nc():
    import concourse.bass as bass  # noqa: F401
    import concourse.mybir as mybir
    import concourse.tile as tile
    from concourse import bacc

    f32 = mybir.dt.float32
    bf16 = mybir.dt.bfloat16
    AF = mybir.ActivationFunctionType

    nc = bacc.Bacc(None, target_bir_lowering=False, debug=False)

    # xt: per-core x[b]^T pre-tiled; odd cores get a seq-roll by 512 so
    # every core's query rows are columns 0:SQ (the host un-rolls attn's
    # k axis on gather; attention output is invariant to k permutation).
    xt = nc.dram_tensor("xt", [P, EC, S], bf16, kind="ExternalInput")
    wqt = nc.dram_tensor("wqt", [P, EC, E], bf16, kind="ExternalInput")
    wkt = nc.dram_tensor("wkt", [P, EC, E], bf16, kind="ExternalInput")
    wvt = nc.dram_tensor("wvt", [P, EC, E], bf16, kind="ExternalInput")
    wot = nc.dram_tensor("wot", [P, EC, E], bf16, kind="ExternalInput")
    wdt = nc.dram_tensor("wdt", [P, EC, F], bf16, kind="ExternalInput")
    bq = nc.dram_tensor("bq", [P, EC], f32, kind="ExternalInput")
    bk = nc.dram_tensor("bk", [P, EC], f32, kind="ExternalInput")
    bfin = nc.dram_tensor("bfin", [P, F], f32, kind="ExternalInput")
    # attn in [head, key, query] orientation, UNNORMALIZED; the host divides
    # by rsums (per-head softmax row sums) and transposes during gather.
    attn = nc.dram_tensor("attn", [H, S, SQ], bf16, kind="ExternalOutput")
    rsums = nc.dram_tensor("rsums", [H, SQ], f32, kind="ExternalOutput")
    out = nc.dram_tensor("out", [SQ, F], f32, kind="ExternalOutput")

    with tile.TileContext(nc) as tc:
        with (
            tc.tile_pool(name="wpool", bufs=1) as wpool,
            tc.tile_pool(name="xpool", bufs=1) as xpool,
            tc.tile_pool(name="acts", bufs=1) as acts,
            tc.tile_pool(name="expp", bufs=2) as expp,
            tc.tile_pool(name="rbp", bufs=2) as rbp,
            tc.tile_pool(name="small", bufs=4) as small,
            tc.tile_pool(name="o2p", bufs=2) as o2p,
            tc.tile_pool(name="psum", bufs=2, space="PSUM") as psum,
            tc.tile_pool(name="spsum", bufs=2, space="PSUM") as spsum,
            tc.tile_pool(name="upsum", bufs=2, space="PSUM") as upsum,
        ):
            def load_tiled(pool, dram, cols, tag, eng=None):
                # split into per-chunk DMAs: more HW-DGE parallelism, and
                # consumers of chunk ec only wait for chunk ec's transfer
                t = pool.tile([P, EC, cols], bf16, tag=tag)
                for ec in range(EC):
                    e = eng if eng is not None else (
                        nc.sync if ec % 2 == 0 else nc.scalar
                    )
                    e.dma_start(t[:, ec], dram[:, ec])
                return t

            # PE warm-up: dummy matmuls with no input deps run during the
            # initial load wait and flip the HAM clock gate to 2.4 GHz
            # before the real matmuls start.
            wdum = wpool.tile([P, 512], bf16, tag="wdum")
            nc.vector.memset(wdum[:], 0.001)
            pdum = psum.tile([P, 512], f32, tag="mm", name="pdum")
            for r in range(24):
                nc.tensor.matmul(
                    pdum[:], wdum[:, 0:P], wdum[:],
                    start=(r == 0), stop=(r == 23),
                )

            wq_sb = load_tiled(wpool, wqt, E, "wq")
            xt_sb = load_tiled(xpool, xt, S, "xt")
            wk_sb = load_tiled(wpool, wkt, E, "wk")
            wv_sb = load_tiled(wpool, wvt, E, "wv")

            bq_sb = wpool.tile([P, EC], f32, tag="bq")
            nc.scalar.dma_start(bq_sb[:], bq[:])
            bk_sb = wpool.tile([P, EC], f32, tag="bk")
            nc.scalar.dma_start(bk_sb[:], bk[:])
            bf_sb = wpool.tile([P, F], f32, tag="bfin")
            nc.scalar.dma_start(bf_sb[:], bfin[:])

            # ---- V first (with ones columns), then a fused per-pair
            # pipeline: QT/KT chunk j feed head pair j immediately, so the
            # ACT exp stream overlaps the projection matmuls ----
            # V with a ones column interleaved per head: [k, h*65+j] with
            # col 64 of each head == 1.0 (gives rowsum in the av matmul).
            vones = acts.tile([P, EC, H * VW], bf16, tag="vones")
            nc.gpsimd.memset(vones[:], 1.0)
            for sc in range(EC):
                for v2 in range(2):
                    ps = psum.tile([P, 512], f32, tag="mm")
                    for ec in range(EC):
                        nc.tensor.matmul(
                            ps[:],
                            xt_sb[:, ec, sc * P:(sc + 1) * P],
                            wv_sb[:, ec, v2 * 512:(v2 + 1) * 512],
                            start=(ec == 0),
                            stop=(ec == EC - 1),
                        )
                    dst = vones[:, sc, :].rearrange(
                        "p (h j) -> p h j", j=VW
                    )[:, v2 * 8:(v2 + 1) * 8, 0:D]
                    src = ps[:].rearrange("p (h j) -> p h j", j=D)
                    nc.vector.tensor_copy(dst, src)

            # phase-3 weights in their own slots; transfers overlap the
            # attention phase.
            wo_sb = load_tiled(wpool, wot, E, "wo")
            wd_sb = load_tiled(wpool, wdt, F, "wd")

            # ---- Attention, software-pipelined by one pair: while ACT
            # streams pair j's exp, the PE runs pair j+1's QT/KT chunks
            # and pair j's av. ----
            qt = acts.tile([P, EC, SQ], bf16, tag="qt")
            kt = acts.tile([P, EC, S], bf16, tag="kt")
            ut = acts.tile([P, EC, SQ], bf16, tag="ut")

            def proj_qk(j):
                ps = psum.tile([P, 512], f32, tag="mm")
                for ec in range(EC):
                    nc.tensor.matmul(
                        ps[:],
                        wq_sb[:, ec, j * P:(j + 1) * P],
                        xt_sb[:, ec, 0:SQ],
                        start=(ec == 0),
                        stop=(ec == EC - 1),
                    )
                nc.vector.tensor_scalar_add(
                    qt[:, j, :], ps[:], bq_sb[:, j:j + 1]
                )
                for k2 in range(2):
                    ps = psum.tile([P, 512], f32, tag="mm")
                    for ec in range(EC):
                        nc.tensor.matmul(
                            ps[:],
                            wk_sb[:, ec, j * P:(j + 1) * P],
                            xt_sb[:, ec, k2 * 512:(k2 + 1) * 512],
                            start=(ec == 0),
                            stop=(ec == EC - 1),
                        )
                    nc.vector.tensor_scalar_add(
                        kt[:, j, k2 * 512:(k2 + 1) * 512],
                        ps[:],
                        bk_sb[:, j:j + 1],
                    )

            proj_qk(0)
            for j in range(H // 2):
                # both heads of the pair share one exp tile: free dim is
                # [hh*512 + q] so one ACT instruction evicts both heads.
                ex = expp.tile([P, EC, 2 * SQ], bf16, tag="exp")
                for kc in range(EC):
                    pss = spsum.tile([P, 1024], f32, tag="mms")
                    for hh in range(2):
                        pb = hh * 64
                        nc.tensor.matmul(
                            pss[:, hh * 512:(hh + 1) * 512],
                            kt[pb:pb + 64, j, kc * P:(kc + 1) * P],
                            qt[pb:pb + 64, j, :],
                            start=True,
                            stop=True,
                        )
                    nc.scalar.activation(
                        ex[:, kc, :], pss[:], AF.Exp, scale=0.125
                    )
                    # unnormalized attn tiles straight out (the host divides
                    # by rsums); alternating queues
                    for hh in range(2):
                        qs = slice(hh * 512, (hh + 1) * 512)
                        nc.sync.dma_start(
                            attn[2 * j + hh, kc * P:(kc + 1) * P, :],
                            ex[:, kc, qs],
                        )

                # next pair's projections fill the PE while ACT does exp
                if j + 1 < H // 2:
                    proj_qk(j + 1)

                for hh in range(2):
                    h = 2 * j + hh
                    qs = slice(hh * 512, (hh + 1) * 512)
                    psu = upsum.tile([VW, 512], f32, tag="u")
                    for kc in range(EC):
                        nc.tensor.matmul(
                            psu[:],
                            vones[:, kc, h * VW:(h + 1) * VW],
                            ex[:, kc, qs],
                            start=(kc == 0),
                            stop=(kc == EC - 1),
                        )
                    # evict fast so the PSUM bank frees for the next head
                    u_raw = rbp.tile([D, 512], f32, tag="uraw")
                    nc.vector.tensor_copy(u_raw[:], psu[0:D, :])
                    rrow = small.tile([1, 512], f32, tag="rrow")
                    nc.vector.tensor_copy(rrow[:], psu[D:D + 1, :])
                    nc.sync.dma_start(rsums[h:h + 1, :], rrow[:])
                    # broadcast rowsum, then fast approximate reciprocal
                    rsb = rbp.tile([D, 512], f32, tag="rsb")
                    nc.gpsimd.partition_broadcast(rsb[:], rrow[:], channels=D)
                    rb32 = rbp.tile([D, 512], f32, tag="rb32")
                    nc.vector.reciprocal_approx_fast(rb32[:], rsb[:])
                    # normalized per-head output rows into UT
                    nc.vector.tensor_mul(
                        ut[hh * 64:hh * 64 + 64, j, :],
                        u_raw[:],
                        rb32[:],
                    )

            # ---- Phase 3: output projections ----
            o1 = acts.tile([P, EC, SQ], bf16, tag="o1")
            for oc in range(EC):
                ps = psum.tile([P, 512], f32, tag="mm")
                for ec in range(EC):
                    nc.tensor.matmul(
                        ps[:],
                        wo_sb[:, ec, oc * P:(oc + 1) * P],
                        ut[:, ec, :],
                        start=(ec == 0),
                        stop=(ec == EC - 1),
                    )
                nc.vector.tensor_copy(o1[:, oc, :], ps[:])

            for qc in range(4):
                ps = psum.tile([P, 512], f32, tag="mm")
                for oc in range(EC):
                    nc.tensor.matmul(
                        ps[:],
                        o1[:, oc, qc * P:(qc + 1) * P],
                        wd_sb[:, oc, :],
                        start=(oc == 0),
                        stop=(oc == EC - 1),
                    )
                o2 = o2p.tile([P, F], f32, tag="o2")
                nc.vector.tensor_add(o2[:], ps[:], bf_sb[:])
                nc.sync.dma_start(out[qc * P:(qc + 1) * P, :], o2[:])

    nc.compile()
    return nc


def _get_nc():
    if "nc" not in _CACHE:
        _CACHE["nc"] = _build_nc()
    return _CACHE["nc"]


def _prep_in_maps(x, Wq, bq, Wk, bk, Wv, bv, Wo, bo, Wd, bd):
    x = np.asarray(x, np.float32)
    Wq, Wk, Wv, Wo, Wd = (np.asarray(w, np.float32) for w in (Wq, Wk, Wv, Wo, Wd))
    bq, bk, bv, bo, bd = (np.asarray(b, np.float32) for b in (bq, bk, bv, bo, bd))

    def tile_pmaj(a):
        # [E, cols] -> [P, EC, cols] with row (o*P + p) at [p, o]
        return np.ascontiguousarray(
            a.reshape(EC, P, a.shape[1]).transpose(1, 0, 2)
        ).astype(BF16)

    wqt = tile_pmaj(Wq.T)
    wkt = tile_pmaj(Wk.T)
    wvt = tile_pmaj(Wv.T)
    wot = tile_pmaj(Wo.T)
    wdt = tile_pmaj(Wd.T)
    bq_t = np.ascontiguousarray(bq.reshape(EC, P).T)
    bk_t = np.ascontiguousarray(bk.reshape(EC, P).T)
    b_final = (
        bd.astype(np.float64)
        + Wd.astype(np.float64) @ (bo.astype(np.float64)
                                   + Wo.astype(np.float64) @ bv.astype(np.float64))
    ).astype(np.float32)
    bfin = np.tile(b_final[None, :], (P, 1))

    in_maps = []
    xt_cache = {}
    for c in range(NCORES):
        b, half = c // 2, c % 2
        if (b, half) not in xt_cache:
            xb = x[b].T
            if half:
                xb = np.roll(xb, -SQ, axis=1)
            xt_cache[(b, half)] = tile_pmaj(xb)
        in_maps.append({
            "xt": xt_cache[(b, half)],
            "wqt": wqt, "wkt": wkt, "wvt": wvt, "wot": wot, "wdt": wdt,
            "bq": bq_t, "bk": bk_t, "bfin": bfin,
        })
    return in_maps


def _assemble(results):
    out_full = np.empty((B, S, F), np.float32)
    attn_full = np.empty((B, H, S, S), np.float32)
    for c, res in enumerate(results):
        b, half = c // 2, c % 2
        rows = slice(half * SQ, (half + 1) * SQ)
        out_full[b, rows, :] = res["out"]
        # res["attn"] is [H, k, q] unnormalized (k rolled by SQ on odd
        # cores); divide by the softmax row sums and transpose to [h, q, k].
        a = res["attn"].astype(np.float32) / res["rsums"][:, None, :]
        if half:
            a = np.roll(a, SQ, axis=1)
        attn_full[b, :, rows, :] = a.transpose(0, 2, 1)
    return out_full, attn_full


def run(inputs, trace=False, **spmd_kwargs):
    """Run on the 8 NeuronCores; returns ((out, attn), BassKernelResults)."""
    from concourse.bass_utils import run_bass_kernel_spmd

    nc = _get_nc()
    in_maps = _prep_in_maps(
        inputs["x"], inputs["Wq"], inputs["bq"], inputs["Wk"], inputs["bk"],
        inputs["Wv"], inputs["bv"], inputs["Wo"], inputs["bo"],
        inputs["Wd"], inputs["bd"],
    )
    br = run_bass_kernel_spmd(
        nc, in_maps, core_ids=list(range(NCORES)), trace=trace, **spmd_kwargs
    )
    return _assemble(br.results), br


def kernel(**inputs):
    (out_full, attn_full), _ = run(inputs, trace=False)
    return out_full, attn_full


# revision 20
# speedup vs baseline: 1.3415x; 1.3415x over previous
# Multi-head attention kernel for 8 Trainium2 NeuronCores.
#
# Problem: x[4,1024,1024] -> Q/K/V proj (16 heads, d=64) -> softmax(QK^T/8) ->
#          attn@V -> Wo -> Wd, returns (out[4,1024,512], attn[4,16,1024,1024]).
#
# Sharding: pure SPMD, no collectives. Core c handles batch b=c//2 and query
# rows [half*512, half*512+512) with half=c%2. Each core computes K/V for the
# full sequence of its batch (duplicated across the pair - cheaper than a
# cross-core exchange), Q only for its own rows.
#
# Dataflow (per core, all matmuls bf16 inputs, fp32 PSUM). Everything is
# feature-major ("transposed") so the TensorE contraction dim always sits on
# partitions and no on-chip transpose is ever needed:
#   QT[o,q]   = Wq @ x^T (+bq)
#   KT[o,k]   = Wk @ x^T (+bk)
#   Vones[k,·]= x @ Wv^T, interleaved with a ones column per head
#               (bv is folded into the final bias)
#   expT_h    = exp((KT_h^T @ QT_h) / 8)   [k,q] orientation, unnormalized
#   U_h[d,q] | rowsum[q] = Vones_h^T @ expT_h   (ones column -> rowsum row)
#   attn_h    = expT_h * bcast(1/rowsum)   (bf16, written [k,q]; the host
#               transposes per-core attn back to [q,k] during gather)
#   UT[o,q]   = U_h * bcast(1/rowsum)      (per-head normalized)
#   O1T[o,q]  = Wo @ UT
#   out[q,f]  = O1T^T @ Wd^T + b_final     (q-major: contiguous output DMA)
# where b_final = bd + Wd@(bo + Wo@bv) is computed on host.

import numpy as np
import ml_dtypes

B, S, E = 4, 1024, 1024
H, D = 16, 64
F = 512
NCORES = 8
P = 128
SQ = S // 2  # 512 query rows per core
EC = E // P  # 8 feature chunks
VW = D + 1   # V columns per head incl. the ones column

BF16 = ml_dtypes.bfloat16

_CACHE = {}


def _build_nc():
    import concourse.bass as bass  # noqa: F401
    import concourse.mybir as mybir
    import concourse.tile as tile
    from concourse import bacc

    f32 = mybir.dt.float32
    bf16 = mybir.dt.bfloat16
    AF = mybir.ActivationFunctionType

    nc = bacc.Bacc(None, target_bir_lowering=False, debug=False)

    # xt: per-core x[b]^T pre-tiled; odd cores get a seq-roll by 512 so
    # every core's query rows are columns 0:SQ (the host un-rolls attn's
    # k axis on gather; attention output is invariant to k permutation).
    xt = nc.dram_tensor("xt", [P, EC, S], bf16, kind="ExternalInput")
    wqt = nc.dram_tensor("wqt", [P, EC, E], bf16, kind="ExternalInput")
    wkt = nc.dram_tensor("wkt", [P, EC, E], bf16, kind="ExternalInput")
    wvt = nc.dram_tensor("wvt", [P, EC, E], bf16, kind="ExternalInput")
    # wct = (Wd @ Wo)^T pre-tiled: Wo and Wd fold into one matmul
    wct = nc.dram_tensor("wct", [P, EC, F], bf16, kind="ExternalInput")
    bq = nc.dram_tensor("bq", [P, EC], f32, kind="ExternalInput")
    bk = nc.dram_tensor("bk", [P, EC], f32, kind="ExternalInput")
    bfin = nc.dram_tensor("bfin", [P, F], f32, kind="ExternalInput")
    # attn in [head, key, query] orientation, UNNORMALIZED; the host divides
    # by rsums (per-head softmax row sums) and transposes during gather.
    attn = nc.dram_tensor("attn", [H, S, SQ], bf16, kind="ExternalOutput")
    rsums = nc.dram_tensor("rsums", [H, SQ], f32, kind="ExternalOutput")
    out = nc.dram_tensor("out", [SQ, F], f32, kind="ExternalOutput")

    with tile.TileContext(nc) as tc:
        with (
            tc.tile_pool(name="wpool", bufs=1) as wpool,
            tc.tile_pool(name="xpool", bufs=1) as xpool,
            tc.tile_pool(name="acts", bufs=1) as acts,
            tc.tile_pool(name="expp", bufs=2) as expp,
            tc.tile_pool(name="rbp", bufs=2) as rbp,
            tc.tile_pool(name="small", bufs=4) as small,
            tc.tile_pool(name="o2p", bufs=2) as o2p,
            tc.tile_pool(name="psum", bufs=2, space="PSUM") as psum,
            tc.tile_pool(name="spsum", bufs=2, space="PSUM") as spsum,
            tc.tile_pool(name="upsum", bufs=2, space="PSUM") as upsum,
        ):
            def load_tiled(pool, dram, cols, tag, eng=None):
                # split into per-chunk DMAs: more HW-DGE parallelism, and
                # consumers of chunk ec only wait for chunk ec's transfer
                t = pool.tile([P, EC, cols], bf16, tag=tag)
                for ec in range(EC):
                    e = eng if eng is not None else (
                        nc.sync if ec % 2 == 0 else nc.scalar
                    )
                    e.dma_start(t[:, ec], dram[:, ec])
                return t

            # PE warm-up: dummy matmuls with no input deps run during the
            # initial load wait and flip the HAM clock gate to 2.4 GHz
            # before the real matmuls start.
            wdum = wpool.tile([P, 512], bf16, tag="wdum")
            nc.vector.memset(wdum[:], 0.001)
            pdum = psum.tile([P, 512], f32, tag="mm", name="pdum")
            for r in range(24):
                nc.tensor.matmul(
                    pdum[:], wdum[:, 0:P], wdum[:],
                    start=(r == 0), stop=(r == 23),
                )

            xt_sb = load_tiled(xpool, xt, S, "xt")
            wv_sb = load_tiled(wpool, wvt, E, "wv")
            wq_sb = load_tiled(wpool, wqt, E, "wq")
            wk_sb = load_tiled(wpool, wkt, E, "wk")

            bq_sb = wpool.tile([P, EC], f32, tag="bq")
            nc.scalar.dma_start(bq_sb[:], bq[:])
            bk_sb = wpool.tile([P, EC], f32, tag="bk")
            nc.scalar.dma_start(bk_sb[:], bk[:])
            bf_sb = wpool.tile([P, F], f32, tag="bfin")
            nc.scalar.dma_start(bf_sb[:], bfin[:])

            # ---- V first (with ones columns), then a fused per-pair
            # pipeline: QT/KT chunk j feed head pair j immediately, so the
            # ACT exp stream overlaps the projection matmuls ----
            # V with a ones column interleaved per head: [k, h*65+j] with
            # col 64 of each head == 1.0 (gives rowsum in the av matmul).
            vones = acts.tile([P, EC, H * VW], bf16, tag="vones")
            nc.gpsimd.memset(vones[:], 1.0)
            for sc in range(EC):
                for v2 in range(2):
                    ps = psum.tile([P, 512], f32, tag="mm")
                    for ec in range(EC):
                        nc.tensor.matmul(
                            ps[:],
                            xt_sb[:, ec, sc * P:(sc + 1) * P],
                            wv_sb[:, ec, v2 * 512:(v2 + 1) * 512],
                            start=(ec == 0),
                            stop=(ec == EC - 1),
                        )
                    dst = vones[:, sc, :].rearrange(
                        "p (h j) -> p h j", j=VW
                    )[:, v2 * 8:(v2 + 1) * 8, 0:D]
                    src = ps[:].rearrange("p (h j) -> p h j", j=D)
                    nc.vector.tensor_copy(dst, src)

            # output-stage weight in its own slot; transfer overlaps the
            # attention phase.
            wc_sb = load_tiled(wpool, wct, F, "wc")

            # ---- Attention, software-pipelined by one pair: while ACT
            # streams pair j's exp, the PE runs pair j+1's QT/KT chunks
            # and pair j's av. ----
            qt = acts.tile([P, EC, SQ], bf16, tag="qt")
            kt = acts.tile([P, EC, S], bf16, tag="kt")
            ut = acts.tile([P, EC, SQ], bf16, tag="ut")

            def proj_qk(j):
                ps = psum.tile([P, 512], f32, tag="mm")
                for ec in range(EC):
                    nc.tensor.matmul(
                        ps[:],
                        wq_sb[:, ec, j * P:(j + 1) * P],
                        xt_sb[:, ec, 0:SQ],
                        start=(ec == 0),
                        stop=(ec == EC - 1),
                    )
                nc.vector.tensor_scalar_add(
                    qt[:, j, :], ps[:], bq_sb[:, j:j + 1]
                )
                for k2 in range(2):
                    ps = psum.tile([P, 512], f32, tag="mm")
                    for ec in range(EC):
                        nc.tensor.matmul(
                            ps[:],
                            wk_sb[:, ec, j * P:(j + 1) * P],
                            xt_sb[:, ec, k2 * 512:(k2 + 1) * 512],
                            start=(ec == 0),
                            stop=(ec == EC - 1),
                        )
                    nc.vector.tensor_scalar_add(
                        kt[:, j, k2 * 512:(k2 + 1) * 512],
                        ps[:],
                        bk_sb[:, j:j + 1],
                    )

            proj_qk(0)
            for j in range(H // 2):
                # both heads of the pair share one exp tile: free dim is
                # [hh*512 + q] so one ACT instruction evicts both heads.
                ex = expp.tile([P, EC, 2 * SQ], bf16, tag="exp")
                for kc in range(EC):
                    pss = spsum.tile([P, 1024], f32, tag="mms")
                    for hh in range(2):
                        pb = hh * 64
                        nc.tensor.matmul(
                            pss[:, hh * 512:(hh + 1) * 512],
                            kt[pb:pb + 64, j, kc * P:(kc + 1) * P],
                            qt[pb:pb + 64, j, :],
                            start=True,
                            stop=True,
                        )
                    nc.scalar.activation(
                        ex[:, kc, :], pss[:], AF.Exp, scale=0.125
                    )
                    # unnormalized attn tiles straight out (the host divides
                    # by rsums); alternating queues
                    for hh in range(2):
                        qs = slice(hh * 512, (hh + 1) * 512)
                        nc.sync.dma_start(
                            attn[2 * j + hh, kc * P:(kc + 1) * P, :],
                            ex[:, kc, qs],
                        )

                # next pair's projections fill the PE while ACT does exp
                if j + 1 < H // 2:
                    proj_qk(j + 1)

                for hh in range(2):
                    h = 2 * j + hh
                    qs = slice(hh * 512, (hh + 1) * 512)
                    psu = upsum.tile([VW, 512], f32, tag="u")
                    for kc in range(EC):
                        nc.tensor.matmul(
                            psu[:],
                            vones[:, kc, h * VW:(h + 1) * VW],
                            ex[:, kc, qs],
                            start=(kc == 0),
                            stop=(kc == EC - 1),
                        )
                    # evict fast so the PSUM bank frees for the next head
                    u_raw = rbp.tile([D, 512], f32, tag="uraw")
                    nc.vector.tensor_copy(u_raw[:], psu[0:D, :])
                    rrow = small.tile([1, 512], f32, tag="rrow")
                    nc.vector.tensor_copy(rrow[:], psu[D:D + 1, :])
                    nc.sync.dma_start(rsums[h:h + 1, :], rrow[:])
                    # broadcast rowsum, then fast approximate reciprocal
                    rsb = rbp.tile([D, 512], f32, tag="rsb")
                    nc.gpsimd.partition_broadcast(rsb[:], rrow[:], channels=D)
                    rb32 = rbp.tile([D, 512], f32, tag="rb32")
                    nc.vector.reciprocal_approx_fast(rb32[:], rsb[:])
                    # normalized per-head output rows into UT
                    nc.vector.tensor_mul(
                        ut[hh * 64:hh * 64 + 64, j, :],
                        u_raw[:],
                        rb32[:],
                    )

            # ---- Output: out[q,f] = UT^T @ Wc^T + b_final ----
            for qc in range(4):
                ps = psum.tile([P, 512], f32, tag="mm")
                for ec in range(EC):
                    nc.tensor.matmul(
                        ps[:],
                        ut[:, ec, qc * P:(qc + 1) * P],
                        wc_sb[:, ec, :],
                        start=(ec == 0),
                        stop=(ec == EC - 1),
                    )
                o2 = o2p.tile([P, F], f32, tag="o2")
                nc.vector.tensor_add(o2[:], ps[:], bf_sb[:])
                nc.sync.dma_start(out[qc * P:(qc + 1) * P, :], o2[:])

    nc.compile()
    return nc


def _get_nc():
    if "nc" not in _CACHE:
        _CACHE["nc"] = _build_nc()
    return _CACHE["nc"]


def _prep_in_maps(x, Wq, bq, Wk, bk, Wv, bv, Wo, bo, Wd, bd):
    x = np.asarray(x, np.float32)
    Wq, Wk, Wv, Wo, Wd = (np.asarray(w, np.float32) for w in (Wq, Wk, Wv, Wo, Wd))
    bq, bk, bv, bo, bd = (np.asarray(b, np.float32) for b in (bq, bk, bv, bo, bd))

    def tile_pmaj(a):
        # [E, cols] -> [P, EC, cols] with row (o*P + p) at [p, o]
        return np.ascontiguousarray(
            a.reshape(EC, P, a.shape[1]).transpose(1, 0, 2)
        ).astype(BF16)

    wqt = tile_pmaj(Wq.T)
    wkt = tile_pmaj(Wk.T)
    wvt = tile_pmaj(Wv.T)
    wct = tile_pmaj(
        (Wd.astype(np.float64) @ Wo.astype(np.float64)).T.astype(np.float32)
    )
    bq_t = np.ascontiguousarray(bq.reshape(EC, P).T)
    bk_t = np.ascontiguousarray(bk.reshape(EC, P).T)
    b_final = (
        bd.astype(np.float64)
        + Wd.astype(np.float64) @ (bo.astype(np.float64)
                                   + Wo.astype(np.float64) @ bv.astype(np.float64))
    ).astype(np.float32)
    bfin = np.tile(b_final[None, :], (P, 1))

    in_maps = []
    xt_cache = {}
    for c in range(NCORES):
        b, half = c // 2, c % 2
        if (b, half) not in xt_cache:
            xb = x[b].T
            if half:
                xb = np.roll(xb, -SQ, axis=1)
            xt_cache[(b, half)] = tile_pmaj(xb)
        in_maps.append({
            "xt": xt_cache[(b, half)],
            "wqt": wqt, "wkt": wkt, "wvt": wvt, "wct": wct,
            "bq": bq_t, "bk": bk_t, "bfin": bfin,
        })
    return in_maps


def _assemble(results):
    out_full = np.empty((B, S, F), np.float32)
    attn_full = np.empty((B, H, S, S), np.float32)
    for c, res in enumerate(results):
        b, half = c // 2, c % 2
        rows = slice(half * SQ, (half + 1) * SQ)
        out_full[b, rows, :] = res["out"]
        # res["attn"] is [H, k, q] unnormalized (k rolled by SQ on odd
        # cores); divide by the softmax row sums and transpose to [h, q, k].
        a = res["attn"].astype(np.float32) / res["rsums"][:, None, :]
        if half:
            a = np.roll(a, SQ, axis=1)
        attn_full[b, :, rows, :] = a.transpose(0, 2, 1)
    return out_full, attn_full


def run(inputs, trace=False, **spmd_kwargs):
    """Run on the 8 NeuronCores; returns ((out, attn), BassKernelResults)."""
    from concourse.bass_utils import run_bass_kernel_spmd

    nc = _get_nc()
    in_maps = _prep_in_maps(
        inputs["x"], inputs["Wq"], inputs["bq"], inputs["Wk"], inputs["bk"],
        inputs["Wv"], inputs["bv"], inputs["Wo"], inputs["bo"],
        inputs["Wd"], inputs["bd"],
    )
    br = run_bass_kernel_spmd(
        nc, in_maps, core_ids=list(range(NCORES)), trace=trace, **spmd_kwargs
    )
    return _assemble(br.results), br


def kernel(**inputs):
    (out_full, attn_full), _ = run(inputs, trace=False)
    return out_full, attn_full
